# revision 85
# baseline (speedup 1.0000x reference)
"""Trainium2 Bass kernel for nn_MHABlock (dense transformer block).

Sharding: data-parallel over batch — 8 cores x 4 batches (2048 tokens/core).
BatchNorm stats are exact via two tiny cross-core AllGathers ([128,2] each)
plus a local 8-way reduce (AllGather models ~15us vs AllReduce ~28us).

Attention uses the "scoresT" formulation (scores [k_tok, q_tok], keys on
partitions) so the exp output feeds attnV directly as the stationary operand.
attnV is computed TRANSPOSED: out[q_part, (head, v)] with a ones-column
prepended per head (softmax denominators land as a free-dim column per head),
so normalization is a per-partition reciprocal + broadcast multiply on DVE —
no PE broadcast matmuls. The normalized [q, hv] tile is flipped back to
[hv, q] by a PE identity-transpose, and the out-projection is one
[hv=128 -> e=128] matmul per q-chunk. Everything dense runs in bf16.

The exp stream (the critical engine) is split: 3/4 of the score tiles use
the Act engine's Exp; 1/4 are computed on DVE as a Schraudolph bf16
exponential (one tensor_scalar writing int16 bits through a bitcast view),
in a separate PSUM tag so the Act pipeline never waits on DVE. The PE
instruction stream is software-pipelined (scores(b) before attnV(b-1)), and
q/k/v projections are interleaved per batch into the pipeline.
"""

import numpy as np

B, N, D_IN, E, H, KD, FF = 32, 512, 2, 128, 8, 16, 512
NCORES = 8
BPC = B // NCORES          # batches per core
T = BPC * N                # 2048 local tokens
NTOK = B * N               # global token count for BN
NORM = 1.0 / np.sqrt(16.0)
EPS = 1e-5

_CACHE = {}
LAST_RESULT = None


def _build_nc():
    import concourse.bass as bass  # noqa: F401
    import concourse.mybir as mybir
    import concourse.tile as tile
    from concourse import bacc

    f32 = mybir.dt.float32
    f32r = mybir.dt.float32r
    bf16 = mybir.dt.bfloat16
    i16 = mybir.dt.int16
    Act = mybir.ActivationFunctionType
    Alu = mybir.AluOpType
    AX = mybir.AxisListType

    nc = bacc.Bacc("TRN2", target_bir_lowering=False, debug=False,
                   enable_asserts=False, num_devices=NCORES)

    # ---- DRAM I/O ----
    d_xT = nc.dram_tensor("xT", [D_IN + 1, T], bf16,
                          kind="ExternalInput").ap()
    d_We1 = nc.dram_tensor("We1", [D_IN + 1, E], bf16,
                           kind="ExternalInput").ap()
    d_WqQ = nc.dram_tensor("WqQ", [D_IN + 1, 256], bf16,
                           kind="ExternalInput").ap()
    d_WkQ = nc.dram_tensor("WkQ", [D_IN + 1, 256], bf16,
                           kind="ExternalInput").ap()
    d_WvI = nc.dram_tensor("WvI", [D_IN + 1, 128], bf16,
                           kind="ExternalInput").ap()
    d_WoP = nc.dram_tensor("WoP", [128, 128], bf16, kind="ExternalInput").ap()
    d_fW1 = nc.dram_tensor("fW1", [E, FF], bf16, kind="ExternalInput").ap()
    d_fW2 = nc.dram_tensor("fW2", [128, 512], bf16, kind="ExternalInput").ap()
    d_vecs = nc.dram_tensor("vecs", [128, 12], f32, kind="ExternalInput").ap()
    d_ident = nc.dram_tensor("ident", [128, 128], bf16,
                             kind="ExternalInput").ap()
    d_yT = nc.dram_tensor("yT", [E, T], f32, kind="ExternalOutput").ap()

    RG = [list(range(NCORES))]

    with tile.TileContext(nc) as tc:
        with tc.sbuf_pool(name="sb", bufs=1) as sb, \
             tc.psum_pool(name="ps", bufs=1) as ps, \
             tc.tile_pool(name="dr", bufs=1, space="DRAM") as dr:

            def P(shape, dt, name):  # persistent tile
                return sb.tile(shape, dt, name=name, tag=name, bufs=1)

            xT = P([D_IN + 1, T], bf16, "xT_sb")
            We1_sb = P([D_IN + 1, E], bf16, "We1_sb")
            WqQ_sb = P([D_IN + 1, 256], bf16, "WqQ_sb")
            WkQ_sb = P([D_IN + 1, 256], bf16, "WkQ_sb")
            WvI_sb = P([D_IN + 1, 128], bf16, "WvI_sb")
            WoP_sb = P([128, 128], bf16, "WoP_sb")
            fW1_sb = P([128, FF], bf16, "fW1_sb")
            fW2_sb = P([128, 512], bf16, "fW2_sb")
            vecs_sb = P([128, 12], f32, "vecs_sb")
            ident_sb = P([128, 128], bf16, "ident_sb")

            H0T = P([128, T], f32, "H0T")
            qT = [P([128, T], bf16, f"qT{g}") for g in range(2)]
            kT = [P([128, T], bf16, f"kT{g}") for g in range(2)]
            V_aug = P([128, 16 * 136], bf16, "V_aug")
            h1b = P([128, T], bf16, "h1b")
            fW1s = P([128, FF], bf16, "fW1s")
            rbias = P([128, 4], f32, "rbias")
            shb = P([128, 1], bf16, "shb")
            h2T = [P([128, T], bf16, f"h2T{qf}") for qf in range(4)]
            yT = P([128, T], f32, "yT_sb")
            sq = P([128, T], f32, "sq")
            st1 = P([128, 32], f32, "st1")  # per-qc BN1 [sums | sqsums]
            st2 = P([128, 8], f32, "st2")   # per-chunk BN2 [sums | sqsums]
            stf1 = P([128, 2], f32, "stf1")
            stf2 = P([128, 2], f32, "stf2")
            gst1 = P([128, 2], f32, "gst1")
            gst2 = P([128, 2], f32, "gst2")
            bn1s = P([128, 6], f32, "bn1s")
            bn2s = P([128, 6], f32, "bn2s")

            # ---- load inputs: q/k weights + first x chunk lead (HWDGE
            # serializes DMA issue at ~625ns each — the scores path must not
            # queue behind the FFN weights) ----
            nc.sync.dma_start(WqQ_sb[:], d_WqQ)
            nc.sync.dma_start(WkQ_sb[:], d_WkQ)
            nc.sync.dma_start(xT[:, 0:512], d_xT[:, 0:512])
            nc.sync.dma_start(WvI_sb[:], d_WvI)
            for c in range(1, 4):
                nc.sync.dma_start(xT[:, 512 * c:512 * (c + 1)],
                                  d_xT[:, 512 * c:512 * (c + 1)])
            nc.sync.dma_start(We1_sb[:], d_We1)
            nc.sync.dma_start(WoP_sb[:], d_WoP)
            nc.sync.dma_start(ident_sb[:], d_ident)
            nc.sync.dma_start(vecs_sb[:], d_vecs)
            nc.sync.dma_start(fW1_sb[:], d_fW1)
            nc.sync.dma_start(fW2_sb[:], d_fW2)
            nc.gpsimd.memset(V_aug[:], 0.0)
            # ones column per (token-chunk, head) 17-block
            va_ones = V_aug.rearrange("p (t h w) -> p (t h) w", t=16, h=8)
            nc.gpsimd.memset(va_ones[:, :, 0:1], 1.0)

            # ---- PE p-state warmup: dummy matmuls on a memset tile while
            # the input DMAs land, so the real q/k/score matmuls start at
            # full clock (cold PE runs 3.7x slower until ~3us of activity)
            dum = P([128, 512], bf16, "dum")
            nc.vector.memset(dum[:], 0.0)
            for w in range(3):
                pw = ps.tile([128, 512], f32, tag="mm", bufs=2,
                             name=f"warm{w}")
                nc.tensor.matmul(pw[:], lhsT=dum[:, 0:128], rhs=dum[:],
                                 start=True, stop=True)

            # ---- embedding h0 = [x;1] @ [We1;be1] (E-major, bf16), emitted
            # after qkproj(0) so it never delays the first scores ----
            def embed():
                for c in range(4):
                    pm = ps.tile([128, 512], f32, tag="mm", bufs=2,
                                 name=f"pm_e{c}")
                    nc.tensor.matmul(pm[:], lhsT=We1_sb[:],
                                     rhs=xT[:, 512 * c:512 * (c + 1)],
                                     start=True, stop=True)
                    nc.vector.tensor_copy(H0T[:, 512 * c:512 * (c + 1)],
                                          pm[:])

            # ---- Phase B helpers: q/k and v projections, emitted per-batch
            # chunk and interleaved with the attention pipeline ----
            def qkproj(c):
                for g in range(2):
                    pq = ps.tile([128, 512], f32, tag="mm", bufs=2,
                                 name=f"pq{g}{c}")
                    nc.tensor.matmul(pq[:], lhsT=WqQ_sb[:, 128 * g:128 * (g + 1)],
                                     rhs=xT[:, 512 * c:512 * (c + 1)],
                                     start=True, stop=True)
                    nc.vector.tensor_copy(qT[g][:, 512 * c:512 * (c + 1)], pq[:])
                    pk = ps.tile([128, 512], f32, tag="mm", bufs=2,
                                 name=f"pk{g}{c}")
                    nc.tensor.matmul(pk[:], lhsT=WkQ_sb[:, 128 * g:128 * (g + 1)],
                                     rhs=xT[:, 512 * c:512 * (c + 1)],
                                     start=True, stop=True)
                    nc.vector.tensor_copy(kT[g][:, 512 * c:512 * (c + 1)],
                                          pk[:])

            def vproj(c):
                # v projection into V_aug (token-major 17-blocks +ones)
                for t in range(4 * c, 4 * (c + 1)):
                    pv = ps.tile([128, 128], f32, tag="mm", bufs=2,
                                 name=f"pv{t}")
                    nc.tensor.matmul(pv[:], lhsT=xT[:, 128 * t:128 * (t + 1)],
                                     rhs=WvI_sb[:], start=True, stop=True)
                    dst = V_aug[:, 136 * t:136 * (t + 1)]
                    dst = dst.rearrange("p (h w) -> p h w", h=8)[:, :, 1:17]
                    src = pv.rearrange("p (h w) -> p h w", h=8)
                    nc.vector.tensor_copy(dst, src)

            # ---- Phase C: attention (software-pipelined: the PE stream
            # emits scores(b) BEFORE attnV(b-1) so attnV stalls never starve
            # the Act engine's exp stream) ----
            exBs = {}

            def scores_exp(b):
                exB = []
                for kc in range(4):
                    exK = sb.tile([128, 4096], bf16, tag=f"ex{kc}", bufs=2,
                                  name=f"ex{b}{kc}")
                    exB.append(exK)
                    for hp in range(4):
                        if hp == 3:
                            # Schraudolph bf16 exp on DVE (bitcast int16):
                            # bits = round(s*NORM*128/ln2 + (127*128 - 7.4)).
                            # Own psum tag so the Act exp slot rotation never
                            # waits on the (busier) DVE.
                            for j in range(2):
                                h = 2 * hp + j
                                g, hh = h // 4, h % 4
                                scj = ps.tile([128, 512], f32, tag="mm",
                                              bufs=2, name=f"scj{b}{kc}{j}")
                                nc.tensor.matmul(
                                    scj[:],
                                    lhsT=kT[g][32 * hh:32 * (hh + 1),
                                               512 * b + 128 * kc:
                                               512 * b + 128 * (kc + 1)],
                                    rhs=qT[g][32 * hh:32 * (hh + 1),
                                              512 * b:512 * (b + 1)],
                                    start=True, stop=True,
                                    tile_position=(32 * hh, 0))
                                nc.vector.tensor_scalar(
                                    exK[:, 1024 * hp + 512 * j:
                                        1024 * hp + 512 * (j + 1)].bitcast(i16),
                                    scj[:],
                                    float(NORM * 128.0 / np.log(2.0)),
                                    float(127.0 * 128.0 - 7.4),
                                    op0=Alu.mult, op1=Alu.add)
                            continue
                        scp = ps.tile([128, 1024], f32, tag="sc", bufs=2,
                                      name=f"scp{b}{kc}{hp}")
                        for j in range(2):
                            h = 2 * hp + j
                            g, hh = h // 4, h % 4
                            nc.tensor.matmul(
                                scp[:, 512 * j:512 * (j + 1)],
                                lhsT=kT[g][32 * hh:32 * (hh + 1),
                                           512 * b + 128 * kc:
                                           512 * b + 128 * (kc + 1)],
                                rhs=qT[g][32 * hh:32 * (hh + 1),
                                          512 * b:512 * (b + 1)],
                                start=True, stop=True,
                                tile_position=(32 * hh, 0))
                        nc.scalar.activation(
                            exK[:, 1024 * hp:1024 * (hp + 1)], scp[:],
                            Act.Exp, scale=float(NORM))
                exBs[b] = exB

            def attn_tail(b):
                # attnV transposed: av [128 q, 8h x (1 sum + 16 v)]
                exB = exBs[b]
                h2Tb = sb.tile([128, 512], bf16, tag="h2T", bufs=2,
                               name=f"h2T{b}")
                for qc in range(4):
                    av = ps.tile([128, 136], f32, tag="av", bufs=2,
                                 name=f"av{b}{qc}")
                    for h in range(8):
                        for kc in range(4):
                            tci = 4 * b + kc
                            nc.tensor.matmul(
                                av[:, 17 * h:17 * (h + 1)],
                                lhsT=exB[kc][:, 512 * h + 128 * qc:
                                             512 * h + 128 * (qc + 1)],
                                rhs=V_aug[:, 136 * tci + 17 * h:
                                          136 * tci + 17 * (h + 1)],
                                start=(kc == 0), stop=(kc == 3))
                    av3 = av.rearrange("p (h w) -> p h w", h=8)
                    rec8 = sb.tile([128, 8], f32, tag="rec", bufs=2,
                                   name=f"rec{b}{qc}")
                    nc.vector.reciprocal(rec8[:], av3[:, :, 0:1])
                    h2 = sb.tile([128, 128], bf16, tag="h2", bufs=2,
                                 name=f"h2{b}{qc}")
                    nc.vector.tensor_tensor(
                        h2.rearrange("p (h w) -> p h w", h=8),
                        av3[:, :, 1:17],
                        rec8[:].unsqueeze(2).broadcast_to((128, 8, 16)),
                        op=Alu.mult)
                    # [q, hv] -> [hv, q] via PE transpose (identity matmul)
                    tp = ps.tile([128, 272], bf16, tag="av", bufs=2,
                                 name=f"tp{b}{qc}")
                    nc.tensor.transpose(tp[:, 0:128], h2[:], ident_sb[:])
                    if b == 3:
                        # Act is free after the last exp — keep DVE off the
                        # critical tail
                        nc.scalar.activation(
                            h2Tb[:, 128 * qc:128 * (qc + 1)], tp[:, 0:128],
                            Act.Copy)
                    else:
                        nc.vector.tensor_copy(
                            h2Tb[:, 128 * qc:128 * (qc + 1)], tp[:, 0:128])
                    # out-projection + skip, per q-chunk
                    po = ps.tile([128, 512], f32, tag="mm", bufs=2,
                                 name=f"po{b}{qc}")
                    nc.tensor.matmul(po[:, 0:128], lhsT=WoP_sb[:],
                                     rhs=h2Tb[:, 128 * qc:128 * (qc + 1)],
                                     start=True, stop=True)
                    sl = slice(512 * b + 128 * qc, 512 * b + 128 * (qc + 1))
                    # skip-add + square, both carrying BN1 stat accumulation
                    nc.vector.scalar_tensor_tensor(
                        out=h1b[:, sl], in0=po[:, 0:128], scalar=1.0,
                        in1=H0T[:, sl], op0=Alu.mult, op1=Alu.add,
                        accum_out=st1[:, 4 * b + qc:4 * b + qc + 1])
                    nc.vector.scalar_tensor_tensor(
                        out=sq[:, sl], in0=h1b[:, sl], scalar=1.0,
                        in1=h1b[:, sl], op0=Alu.mult, op1=Alu.mult,
                        accum_out=st1[:, 16 + 4 * b + qc:17 + 4 * b + qc])

            qkproj(0)
            vproj(0)
            scores_exp(0)
            embed()
            for b in range(1, 4):
                qkproj(b)
                vproj(b)
                scores_exp(b)
                attn_tail(b - 1)
            attn_tail(3)

            # ---- BatchNorm helper (exact, cross-core stats) ----
            # AllGather the per-core [128,2] stats, then 8-way reduce locally.
            def batchnorm(src, stf, gst, bns, wcol, bcol, ccname, pre=None):
                if pre is not None:
                    # per-chunk stats already accumulated in pre [128,(2,4)]
                    nc.vector.tensor_reduce(
                        out=stf[:, 0:2],
                        in_=pre[:].rearrange("p (s c) -> p s c", s=2),
                        axis=AX.X, op=Alu.add)
                else:
                    nc.vector.reduce_sum(out=stf[:, 0:1], in_=src[:],
                                         axis=AX.X)
                    nc.scalar.activation(sq[:], src[:], Act.Square,
                                         accum_out=stf[:, 1:2])
                cc_in = dr.tile([128, 2], f32, name=f"{ccname}_in",
                                tag=f"{ccname}_in")
                cc_out = dr.tile([NCORES, 128, 2], f32, addr_space="Shared",
                                 name=f"{ccname}_out", tag=f"{ccname}_out")
                nc.sync.dma_start(cc_in[:], stf[:])
                nc.gpsimd.collective_compute(
                    "AllGather", Alu.bypass, replica_groups=RG,
                    ins=[cc_in[:]], outs=[cc_out[:]])
                # gather back core-outer/stat-inner (8B contiguous elements
                # halve the DMA descriptor count); reduce over cores via a
                # strided view that puts the core axis innermost
                gath = sb.tile([128, NCORES, 2], f32, name=f"{ccname}_g",
                               tag=f"{ccname}_g", bufs=1)
                nc.sync.dma_start(gath[:],
                                  cc_out[:].rearrange("g p c -> p g c"))
                nc.vector.tensor_reduce(
                    out=gst[:, 0:2],
                    in_=gath[:].rearrange("p g c -> p c g"),
                    axis=AX.X, op=Alu.add)
                inv_n = 1.0 / float(NTOK)
                nc.vector.tensor_scalar_mul(bns[:, 0:1], gst[:, 0:1], inv_n)
                nc.vector.tensor_scalar_mul(bns[:, 1:2], gst[:, 1:2], inv_n)
                nc.vector.tensor_mul(bns[:, 4:5], bns[:, 0:1], bns[:, 0:1])
                nc.vector.tensor_sub(bns[:, 1:2], bns[:, 1:2], bns[:, 4:5])
                nc.scalar.activation(bns[:, 5:6], bns[:, 1:2], Act.Sqrt,
                                     bias=vecs_sb[:, 9:10])
                nc.vector.reciprocal(bns[:, 2:3], bns[:, 5:6])
                nc.vector.tensor_mul(bns[:, 2:3], bns[:, 2:3],
                                     vecs_sb[:, wcol:wcol + 1])
                nc.vector.tensor_mul(bns[:, 4:5], bns[:, 0:1], bns[:, 2:3])
                nc.vector.tensor_sub(bns[:, 3:4], vecs_sb[:, bcol:bcol + 1],
                                     bns[:, 4:5])

            # ---- BN1 (h1n kept in bf16 only; apply per-chunk, overlapping
            # the FFN) ----
            batchnorm(h1b, stf1, gst1, bn1s, 1, 2, "cc1", pre=st1)

            # ---- FFN (ffb2 cancels inside BN2); relu split Act/DVE; BN1
            # applies run one chunk ahead so relus never stall; BN2 stats
            # accumulate per-chunk inside the FFN pipeline ----
            nc.vector.tensor_scalar_mul(fW1s[:], fW1_sb[:], bn1s[:, 2:3])
            nc.vector.tensor_copy(shb[:], bn1s[:, 3:4])
            for qf in range(4):
                pb = ps.tile([128, 512], f32, tag="mm", bufs=2,
                             name=f"pb{qf}")
                nc.tensor.matmul(pb[:, 0:1],
                                 lhsT=fW1_sb[:, 128 * qf:128 * (qf + 1)],
                                 rhs=shb[:], start=True, stop=True)
                nc.vector.tensor_add(rbias[:, qf:qf + 1], pb[:, 0:1],
                                     vecs_sb[:, 3 + qf:4 + qf])
            for c in range(4):
                p2w = ps.tile([128, 1024], f32, tag="sc", bufs=2,
                              name=f"p2{c}")
                p2 = p2w[:, 0:512]
                for qf in range(4):
                    pf = ps.tile([128, 512], f32, tag="mm", bufs=2,
                                 name=f"pf{qf}{c}")
                    nc.tensor.matmul(pf[:],
                                     lhsT=fW1s[:, 128 * qf:128 * (qf + 1)],
                                     rhs=h1b[:, 512 * c:512 * (c + 1)],
                                     start=True, stop=True)
                    if qf < 3:
                        nc.scalar.activation(h2T[qf][:, 512 * c:512 * (c + 1)],
                                             pf[:], Act.Relu,
                                             bias=rbias[:, qf:qf + 1])
                    else:
                        nc.vector.tensor_scalar(
                            h2T[qf][:, 512 * c:512 * (c + 1)], pf[:],
                            rbias[:, qf:qf + 1], 0.0,
                            op0=Alu.add, op1=Alu.max)
                for qf in range(4):
                    nc.tensor.matmul(p2,
                                     lhsT=fW2_sb[:, 128 * qf:128 * (qf + 1)],
                                     rhs=h2T[qf][:, 512 * c:512 * (c + 1)],
                                     start=(qf == 0), stop=(qf == 3))
                # y-add and square both carry BN2 stat accumulation (fused)
                nc.vector.scalar_tensor_tensor(
                    out=yT[:, 512 * c:512 * (c + 1)],
                    in0=h1b[:, 512 * c:512 * (c + 1)],
                    scalar=bn1s[:, 2:3], in1=p2,
                    op0=Alu.mult, op1=Alu.add, accum_out=st2[:, c:c + 1])
                nc.vector.scalar_tensor_tensor(
                    out=sq[:, 512 * c:512 * (c + 1)],
                    in0=yT[:, 512 * c:512 * (c + 1)], scalar=1.0,
                    in1=yT[:, 512 * c:512 * (c + 1)],
                    op0=Alu.mult, op1=Alu.mult,
                    accum_out=st2[:, 4 + c:5 + c])

            # ---- BN2 + output ----
            batchnorm(yT, stf2, gst2, bn2s, 7, 8, "cc2", pre=st2)
            for c in range(4):
                if c % 2 == 0:
                    nc.vector.tensor_scalar(
                        sq[:, 512 * c:512 * (c + 1)],
                        yT[:, 512 * c:512 * (c + 1)],
                        bn2s[:, 2:3], bn2s[:, 3:4], op0=Alu.mult, op1=Alu.add)
                else:
                    nc.scalar.activation(
                        sq[:, 512 * c:512 * (c + 1)],
                        yT[:, 512 * c:512 * (c + 1)], Act.Identity,
                        scale=bn2s[:, 2:3], bias=bn2s[:, 3:4])
                nc.sync.dma_start(d_yT[:, 512 * c:512 * (c + 1)],
                                  sq[:, 512 * c:512 * (c + 1)])

    nc.compile()
    return nc


def _host_prep(inputs):
    f = np.float32
    Wq, Wk, Wv, Wo = (np.asarray(inputs[k], f) for k in ("Wq", "Wk", "Wv", "Wo"))
    WqQ = np.zeros((2, E, 128), f)
    WkQ = np.zeros((2, E, 128), f)
    for g in range(2):
        for hh in range(4):
            h = 4 * g + hh
            WqQ[g, :, 32 * hh:32 * hh + 16] = Wq[h]
            WkQ[g, :, 32 * hh:32 * hh + 16] = Wk[h]
    WvI = np.ascontiguousarray(np.transpose(Wv, (1, 0, 2)).reshape(E, H * KD))
    WoP = np.ascontiguousarray(Wo.reshape(H * KD, E))
    # fold the embedding (x @ We1 + be1) into the q/k/v projections: the
    # input is augmented with a ones-row, weights become [We1@W; be1@W]
    We1f = np.asarray(inputs["We1"], f)
    be1f = np.asarray(inputs["be1"], f)
    WqQf = np.concatenate([WqQ[0], WqQ[1]], axis=1)      # [E, 256]
    WkQf = np.concatenate([WkQ[0], WkQ[1]], axis=1)
    WqX = np.vstack([We1f @ WqQf, be1f @ WqQf])          # [3, 256]
    WkX = np.vstack([We1f @ WkQf, be1f @ WkQf])
    WvX = np.vstack([We1f @ WvI, be1f @ WvI])            # [3, 128]
    We3 = np.vstack([We1f, be1f])                        # [3, 128]
    fW2 = np.ascontiguousarray(
        np.asarray(inputs["ffW2"], f).reshape(4, 128, E).transpose(1, 0, 2))
    vecs = np.zeros((128, 12), f)
    vecs[:, 0] = inputs["be1"]
    vecs[:, 1] = inputs["bn1_w"]
    vecs[:, 2] = inputs["bn1_b"]
    vecs[:, 3:7] = np.asarray(inputs["ffb1"], f).reshape(4, 128).T
    vecs[:, 7] = inputs["bn2_w"]
    vecs[:, 8] = inputs["bn2_b"]
    vecs[:, 9] = EPS
    import ml_dtypes
    bf = ml_dtypes.bfloat16
    ident = np.eye(128, dtype=f)
    return {
        "We1": np.ascontiguousarray(We3).astype(bf),
        "WqQ": np.ascontiguousarray(WqX).astype(bf),
        "WkQ": np.ascontiguousarray(WkX).astype(bf),
        "WvI": np.ascontiguousarray(WvX).astype(bf),
        "WoP": WoP.astype(bf),
        "fW1": np.ascontiguousarray(np.asarray(inputs["ffW1"], f)).astype(bf),
        "fW2": np.ascontiguousarray(fW2.reshape(128, 512)).astype(bf), "vecs": vecs,
        "ident": ident.astype(bf),
    }


def _get_runner():
    """Build the sharded jitted executable once and cache it."""
    if "runner" in _CACHE:
        return _CACHE["runner"]
    import jax
    import concourse.mybir as mybir
    from jax.sharding import Mesh, PartitionSpec
    from jax.experimental.shard_map import shard_map
    from concourse.bass2jax import (_bass_exec_p, install_neuronx_cc_hook,
                                    partition_id_tensor)

    if "nc" not in _CACHE:
        _CACHE["nc"] = _build_nc()
    nc = _CACHE["nc"]
    install_neuronx_cc_hook()
    assert nc.dbg_addr is None

    partition_name = (nc.partition_id_tensor.name
                      if nc.partition_id_tensor else None)
    in_names, out_names, out_avals, zero_outs = [], [], [], []
    for alloc in nc.m.functions[0].allocations:
        if not isinstance(alloc, mybir.MemoryLocationSet):
            continue
        name = alloc.memorylocations[0].name
        if alloc.kind == "ExternalInput":
            if name != partition_name:
                in_names.append(name)
        elif alloc.kind == "ExternalOutput":
            shape = tuple(alloc.tensor_shape)
            dtype = mybir.dt.np(alloc.dtype)
            out_names.append(name)
            out_avals.append(jax.core.ShapedArray(shape, dtype))
            zero_outs.append(np.zeros(shape, dtype))
    n_params = len(in_names)
    n_outs = len(out_avals)
    all_in_names = list(in_names) + list(out_names)
    if partition_name is not None:
        all_in_names.append(partition_name)
    donate = tuple(range(n_params, n_params + n_outs))

    def _body(*args):
        operands = list(args)
        if partition_name is not None:
            operands.append(partition_id_tensor())
        outs = _bass_exec_p.bind(
            *operands,
            out_avals=tuple(out_avals),
            in_names=tuple(all_in_names),
            out_names=tuple(out_names),
            lowering_input_output_aliases=(),
            sim_require_finite=True,
            sim_require_nnan=True,
            nc=nc,
        )
        return tuple(outs)

    devices = jax.devices()[:NCORES]
    mesh = Mesh(np.asarray(devices), ("core",))
    in_specs = (PartitionSpec("core"),) * (n_params + n_outs)
    out_specs = (PartitionSpec("core"),) * len(out_names)
    sharded = jax.jit(
        shard_map(_body, mesh=mesh, in_specs=in_specs, out_specs=out_specs,
                  check_rep=False),
        donate_argnums=donate, keep_unused=True)

    def run(in_maps):
        per_core = [[np.asarray(m[name]) for name in in_names]
                    for m in in_maps]
        concat_in = [np.concatenate([per_core[c][i] for c in range(NCORES)],
                                    axis=0) for i in range(n_params)]
        concat_zeros = [np.zeros((NCORES * z.shape[0], *z.shape[1:]), z.dtype)
                        for z in zero_outs]
        out_arrs = sharded(*concat_in, *concat_zeros)
        out_arrs = [np.asarray(a) for a in out_arrs]
        return [{name: out_arrs[i].reshape(NCORES, *out_avals[i].shape)[c]
                 for i, name in enumerate(out_names)}
                for c in range(NCORES)]

    _CACHE["runner"] = run
    return run


def _make_in_maps(inputs):
    import ml_dtypes
    shared = _host_prep(inputs)
    x1 = np.asarray(inputs["x1"], np.float32)
    in_maps = []
    for cidx in range(NCORES):
        m = dict(shared)
        xl = x1[BPC * cidx:BPC * (cidx + 1)].reshape(T, D_IN)
        xa = np.concatenate([xl.T, np.ones((1, T), np.float32)], axis=0)
        m["xT"] = np.ascontiguousarray(xa).astype(ml_dtypes.bfloat16)
        in_maps.append(m)
    return in_maps


def kernel(**inputs):
    run = _get_runner()
    results = run(_make_in_maps(inputs))
    outs = []
    for cidx in range(NCORES):
        yTo = results[cidx]["yT"]          # [E, T]
        outs.append(np.ascontiguousarray(yTo.T).reshape(BPC, N, E))
    return np.concatenate(outs, 0).astype(np.float32)
